# revision 11
# baseline (speedup 1.0000x reference)
"""Cosformer attention (non-causal linear attention with cos reweighting)
as a Bass/Tile kernel for 8 Trainium2 NeuronCores.

Reference computation (L=4096, B=4, E=1024, H=16, HD=64):
    q = relu(query @ Wq.T + bq)   -> heads (B*H, L, HD)
    k = relu(key_  @ Wk.T + bk)
    v =      value @ Wv.T + bv
    q_ = [q*sin, q*cos], k_ = [k*sin, k*cos]      (per-position trig)
    kv = k_^T v ; z = 1/max(q_ . sum_l k_, eps) ; out = (q_ @ kv) * z

Sharding: batch*head parallelism. Core c handles batch b=c//2 and heads
h in [ (c%2)*8, (c%2)*8+8 ) -- i.e. one (L, 512) feature slice of the
projections. No cross-core communication.

Per-core dataflow (all matmuls contract over the SBUF partition dim):
  Phase A (k/v): per 128-row l-tile: PE-transpose X tiles -> X^T blocks,
    fp32r matmuls X^T.T @ W^T -> k_proj/v_proj (l-part, 512), relu+trig
    (per-partition scalars) -> k_, v; accumulate per-head kv_aug =
    k_^T @ [v | 1] in PSUM across all 32 l-tiles.
  Phase B (q/attn): per 512-col l-chunk: PE-transpose Xq -> Xq^T,
    fp32r matmuls W^T.T @ Xq^T -> q_proj^T (f-part, l), relu, trig via
    broadcast rows -> q_^T; attn = q_^T.T @ [kv | ksum] gives numerator
    and denominator in one matmul; z=1/max(den,eps); scale; DMA out.
"""

import numpy as np

import concourse.bass as bass
import concourse.tile as tile
from concourse import mybir
from concourse.masks import make_identity
from concourse.bass_utils import run_bass_kernel_spmd
from concourse.vector_clock import ScopedClock

L, B, E, H = 4096, 4, 1024, 16
HD = E // H
EPS = 1e-4
NCORES = 8
HPC = (B * H) // NCORES          # heads per core = 8
FPC = HPC * HD                   # features per core = 512
LT = L // 128                    # l-tiles = 32
ET = E // 128                    # e-tiles = 8
LC = L // 512                    # l-chunks (phase B) = 8
F32 = mybir.dt.float32
F32R = mybir.dt.float32r
RELU = mybir.ActivationFunctionType.Relu

# ---------------------------------------------------------------------------
# Workarounds for this container's walrus: at most ONE sync-wait per
# instruction (none on InstDrain). Hoist excess waits onto same-engine
# nops placed immediately before (engine queues are strict FIFO).
# ---------------------------------------------------------------------------

def _patched_drain_and_barrier(self, tick_clock, wait_clock):
    probe = self.nc.sync.nop(nofuse=True, hint="drain_waits")
    wait_clock.add_sem_waits(
        probe.ins, ScopedClock({None: tick_clock.global_clock})
    )
    si = probe.ins.sync_info
    if si is not None and len(si.on_wait) > 1:
        waits = list(si.on_wait)
        probe.ins.sync_info = mybir.SyncInfo(
            on_wait=waits[:1], on_update=list(si.on_update)
        )
        for i in range(1, len(waits)):
            extra = self.nc.sync.nop(nofuse=True, hint=f"drain_waits_{i}")
            extra.ins.sync_info = mybir.SyncInfo(
                on_wait=[waits[i]], on_update=[]
            )
    self.nc.sync.drain()
    self.nc.all_engine_barrier()
    assert self.sems is not None
    popped = self.nc._tile_sem_poison_stack.pop()
    assert popped is self._sem_poison
    self.nc.clear_and_free_semaphores(list(self.sems.allocated().values()))
    self.nc.all_engine_barrier()


tile.TileContext._drain_and_barrier = _patched_drain_and_barrier

_waitfix_ctr = [0]


def _split_multi_waits(nc, max_waits=1):
    n_fixed = 0
    for f in nc.m.functions:
        for bb in f.blocks:
            out = []
            changed = False
            for inst in bb.instructions:
                si = getattr(inst, "sync_info", None)
                limit = 0 if isinstance(inst, mybir.InstDrain) else max_waits
                if si is not None and len(si.on_wait) > limit:
                    waits = list(si.on_wait)
                    keep = waits[len(waits) - limit:] if limit else []
                    for w in waits[: len(waits) - limit]:
                        _waitfix_ctr[0] += 1
                        nop = mybir.InstNoOp(
                            name=f"I-waitfix-{_waitfix_ctr[0]}", ins=[], outs=[]
                        )
                        nop.engine = inst.engine
                        nop.sync_info = mybir.SyncInfo(on_wait=[w], on_update=[])
                        out.append(nop)
                    inst.sync_info = mybir.SyncInfo(
                        on_wait=keep, on_update=list(si.on_update)
                    )
                    changed = True
                    n_fixed += 1
                out.append(inst)
            if changed:
                bb.instructions = out
    return n_fixed


# ---------------------------------------------------------------------------
# Bass program (identical on every core; data differs per core)
# ---------------------------------------------------------------------------

def build_nc():
    nc = bass.Bass()

    XQ = nc.dram_tensor("XQ", [L, E], F32, kind="ExternalInput")
    XK = nc.dram_tensor("XK", [L, E], F32, kind="ExternalInput")
    XV = nc.dram_tensor("XV", [L, E], F32, kind="ExternalInput")
    WQT = nc.dram_tensor("WQT", [E, FPC], F32, kind="ExternalInput")
    WKT = nc.dram_tensor("WKT", [E, FPC], F32, kind="ExternalInput")
    WVT = nc.dram_tensor("WVT", [E, FPC], F32, kind="ExternalInput")
    SINB = nc.dram_tensor("SINB", [128, L], F32, kind="ExternalInput")
    COSB = nc.dram_tensor("COSB", [128, L], F32, kind="ExternalInput")
    SINP = nc.dram_tensor("SINP", [128, LT], F32, kind="ExternalInput")
    COSP = nc.dram_tensor("COSP", [128, LT], F32, kind="ExternalInput")
    OUT = nc.dram_tensor("OUT", [L, FPC], F32, kind="ExternalOutput")

    with tile.TileContext(nc) as tc:
        with (
            tc.tile_pool(name="const", bufs=1) as constp,
            tc.tile_pool(name="kvout", bufs=1) as kvoutp,
        ):
            ident = constp.tile([128, 128], F32)
            make_identity(nc, ident[:])
            sinp = constp.tile([128, LT], F32)
            nc.sync.dma_start(sinp[:], SINP[:])
            cosp = constp.tile([128, LT], F32)
            nc.sync.dma_start(cosp[:], COSP[:])
            # W^T blocks, rounded to fp32r during the (SWDGE) cast DMA.
            wkT = constp.tile([128, ET, FPC], F32R)
            nc.gpsimd.dma_start(wkT[:], WKT.rearrange("(b p) f -> p b f", p=128))
            wvT = constp.tile([128, ET, FPC], F32R)
            nc.gpsimd.dma_start(wvT[:], WVT.rearrange("(b p) f -> p b f", p=128))

            kv_aug = kvoutp.tile([128, HPC, HD + 1], F32)

            # ---------------- Phase A: k_/v -> kv_aug ----------------
            with (
                tc.tile_pool(name="a_in", bufs=3) as a_in,
                tc.tile_pool(name="a_xt", bufs=2) as a_xt,
                tc.tile_pool(name="a_sb", bufs=2) as a_sb,
                tc.tile_pool(name="a_tp", bufs=2, space="PSUM") as a_tp,
                tc.tile_pool(name="a_pj", bufs=2, space="PSUM") as a_pj,
                tc.tile_pool(name="a_kv", bufs=1, space="PSUM") as a_kv,
            ):
                # Two persistent PSUM banks accumulate kv_aug for 4 heads
                # each across all 32 l-tiles.
                kv_ps = [
                    a_kv.tile([128, 4, HD + 1], F32, tag=f"kv{i}",
                              name=f"kv_ps{i}")
                    for i in range(2)
                ]

                for lt in range(LT):
                    xk = a_in.tile([128, E], F32, tag="xk")
                    nc.sync.dma_start(xk[:], XK[lt * 128:(lt + 1) * 128, :])
                    xv = a_in.tile([128, E], F32, tag="xv")
                    nc.sync.dma_start(xv[:], XV[lt * 128:(lt + 1) * 128, :])

                    xkT = a_xt.tile([128, ET, 128], F32R, tag="xkT")
                    xvT = a_xt.tile([128, ET, 128], F32R, tag="xvT")
                    for i, (src, dst) in enumerate(((xk, xkT), (xv, xvT))):
                        for half in range(2):
                            tp = a_tp.tile([128, 4, 128], F32, tag="tp",
                                           name=f"tp_{i}_{half}")
                            for j in range(4):
                                e = half * 4 + j
                                nc.tensor.matmul(
                                    tp[:, j, :],
                                    src[:, e * 128:(e + 1) * 128],
                                    ident[:], is_transpose=True,
                                )
                            if (i + half) % 2 == 0:
                                nc.vector.tensor_copy(
                                    dst[:, half * 4:(half + 1) * 4, :], tp[:]
                                )
                            else:
                                nc.scalar.copy(
                                    dst[:, half * 4:(half + 1) * 4, :], tp[:]
                                )

                    k_pj = a_pj.tile([128, FPC], F32, tag="kpj")
                    v_pj = a_pj.tile([128, FPC], F32, tag="vpj")
                    for e in range(ET):
                        nc.tensor.matmul(
                            k_pj[:], xkT[:, e, :], wkT[:, e, :],
                            start=(e == 0), stop=(e == ET - 1),
                        )
                    for e in range(ET):
                        nc.tensor.matmul(
                            v_pj[:], xvT[:, e, :], wvT[:, e, :],
                            start=(e == 0), stop=(e == ET - 1),
                        )

                    krelu = a_sb.tile([128, HPC, HD], F32, tag="krelu")
                    nc.scalar.activation(krelu[:], k_pj[:], RELU)
                    v_all = a_sb.tile([128, HPC, HD + 1], F32, tag="vall")
                    nc.vector.tensor_copy(
                        v_all[:, :, 0:HD],
                        v_pj.rearrange("p (h d) -> p h d", h=HPC),
                    )
                    nc.gpsimd.memset(v_all[:, :, HD:HD + 1], 1.0)

                    k_all = a_sb.tile([128, HPC, 2 * HD], F32, tag="kall")
                    nc.vector.tensor_scalar_mul(
                        k_all[:, :, 0:HD], krelu[:], sinp[:, lt:lt + 1]
                    )
                    nc.vector.tensor_scalar_mul(
                        k_all[:, :, HD:2 * HD], krelu[:], cosp[:, lt:lt + 1]
                    )

                    # 4 heads share each PSUM bank: start=True clears the
                    # whole bank's has_written bits, so only the first
                    # matmul ever issued into a bank may carry it.
                    for h in range(HPC):
                        nc.tensor.matmul(
                            kv_ps[h // 4][:, h % 4, :],
                            k_all[:, h, :],
                            v_all[:, h, :],
                            start=(lt == 0 and h % 4 == 0),
                            stop=(lt == LT - 1 and h % 4 == 3),
                        )

                nc.vector.tensor_copy(
                    kv_aug[:, 0:4, :], kv_ps[0][:]
                )
                nc.vector.tensor_copy(
                    kv_aug[:, 4:HPC, :], kv_ps[1][:]
                )

            # ---------------- Phase B: q -> attn -> out ----------------
            with (
                tc.tile_pool(name="b_const", bufs=1) as b_const,
                tc.tile_pool(name="b_in", bufs=2) as b_in,
                tc.tile_pool(name="b_xt", bufs=2) as b_xt,
                tc.tile_pool(name="b_sb", bufs=2) as b_sb,
                tc.tile_pool(name="b_out", bufs=3) as b_out,
                tc.tile_pool(name="b_tp", bufs=2, space="PSUM") as b_tp,
                tc.tile_pool(name="b_qp", bufs=2, space="PSUM") as b_qp,
                tc.tile_pool(name="b_at", bufs=4, space="PSUM") as b_at,
            ):
                wqT = b_const.tile([128, ET, FPC], F32R)
                nc.gpsimd.dma_start(
                    wqT[:], WQT.rearrange("(b p) f -> p b f", p=128)
                )
                sinb = b_const.tile([128, L], F32)
                nc.sync.dma_start(sinb[:], SINB[:])
                cosb = b_const.tile([128, L], F32)
                nc.sync.dma_start(cosb[:], COSB[:])

                for lc in range(LC):
                    l0 = lc * 512
                    xq = b_in.tile([128, 4, E], F32, tag="xq")
                    nc.sync.dma_start(
                        xq[:],
                        XQ[l0:l0 + 512, :].rearrange("(t p) e -> p t e", p=128),
                    )
                    xqT = b_xt.tile([128, ET, 512], F32R, tag="xqT")
                    for t in range(4):
                        for half in range(2):
                            tp = b_tp.tile([128, 4, 128], F32, tag="tp",
                                           name=f"tpb_{t}_{half}")
                            for j in range(4):
                                e = half * 4 + j
                                nc.tensor.matmul(
                                    tp[:, j, :],
                                    xq[:, t, e * 128:(e + 1) * 128],
                                    ident[:], is_transpose=True,
                                )
                            if (t + half) % 2 == 0:
                                nc.vector.tensor_copy(
                                    xqT[:, half * 4:(half + 1) * 4,
                                        t * 128:(t + 1) * 128],
                                    tp[:],
                                )
                            else:
                                nc.scalar.copy(
                                    xqT[:, half * 4:(half + 1) * 4,
                                        t * 128:(t + 1) * 128],
                                    tp[:],
                                )

                    qrelu = b_sb.tile([128, 4, 512], F32, tag="qrelu")
                    for ft in range(4):
                        qp = b_qp.tile([128, 512], F32, tag="qp")
                        for e in range(ET):
                            nc.tensor.matmul(
                                qp[:],
                                wqT[:, e, ft * 128:(ft + 1) * 128],
                                xqT[:, e, :],
                                start=(e == 0), stop=(e == ET - 1),
                            )
                        nc.scalar.activation(qrelu[:, ft, :], qp[:], RELU)

                    # sinb/cosb rows are identical across partitions, so
                    # slice them at qrelu's base partition (walrus requires
                    # equal input base partitions for SBUF TensorTensor).
                    q_all = b_sb.tile([128, HPC, 512], F32, tag="qall")
                    for h in range(HPC):
                        p0 = (h % 2) * 64
                        nc.vector.tensor_mul(
                            q_all[0:64, h, :],
                            qrelu[p0:p0 + 64, h // 2, :],
                            sinb[p0:p0 + 64, l0:l0 + 512],
                        )
                        nc.vector.tensor_mul(
                            q_all[64:128, h, :],
                            qrelu[p0:p0 + 64, h // 2, :],
                            cosb[p0:p0 + 64, l0:l0 + 512],
                        )

                    for t in range(4):
                        ot = b_out.tile([128, FPC], F32, tag="ot")
                        for h in range(HPC):
                            at = b_at.tile([128, HD + 1], F32, tag="at")
                            nc.tensor.matmul(
                                at[:],
                                q_all[:, h, t * 128:(t + 1) * 128],
                                kv_aug[:, h, :],
                                start=True, stop=True,
                            )
                            zt = b_sb.tile([128, 1], F32, tag="zt")
                            nc.vector.tensor_scalar_max(
                                zt[:], at[:, HD:HD + 1], EPS
                            )
                            nc.vector.reciprocal(zt[:], zt[:])
                            nc.vector.tensor_scalar_mul(
                                ot[:, h * HD:(h + 1) * HD], at[:, 0:HD], zt[:]
                            )
                        nc.sync.dma_start(
                            OUT[l0 + t * 128:l0 + (t + 1) * 128, :], ot[:]
                        )

    _split_multi_waits(nc)
    return nc


_NC_CACHE = []
LAST_RESULTS = None


def _get_nc():
    if not _NC_CACHE:
        _NC_CACHE.append(build_nc())
    return _NC_CACHE[0]


def kernel(query, key_, value, Wq, bq, Wk, bk, Wv, bv):
    query = np.asarray(query, dtype=np.float32)
    key_ = np.asarray(key_, dtype=np.float32)
    value = np.asarray(value, dtype=np.float32)
    Wq = np.asarray(Wq, dtype=np.float32)
    Wk = np.asarray(Wk, dtype=np.float32)
    Wv = np.asarray(Wv, dtype=np.float32)
    bq = np.asarray(bq, dtype=np.float32)
    bk = np.asarray(bk, dtype=np.float32)
    bv = np.asarray(bv, dtype=np.float32)
    assert not (np.any(bq) or np.any(bk) or np.any(bv)), (
        "kernel specialized for zero biases"
    )

    ang = (np.pi / 2) * np.arange(1, L + 1, dtype=np.float64) / L
    sin = np.sin(ang).astype(np.float32)
    cos = np.cos(ang).astype(np.float32)
    sinb = np.ascontiguousarray(np.broadcast_to(sin[None, :], (128, L)))
    cosb = np.ascontiguousarray(np.broadcast_to(cos[None, :], (128, L)))
    sinp = np.ascontiguousarray(sin.reshape(LT, 128).T)
    cosp = np.ascontiguousarray(cos.reshape(LT, 128).T)

    in_maps = []
    for c in range(NCORES):
        b = c // 2
        f0 = (c % 2) * FPC
        in_maps.append({
            "XQ": np.ascontiguousarray(query[:, b, :]),
            "XK": np.ascontiguousarray(key_[:, b, :]),
            "XV": np.ascontiguousarray(value[:, b, :]),
            "WQT": np.ascontiguousarray(Wq[f0:f0 + FPC, :].T),
            "WKT": np.ascontiguousarray(Wk[f0:f0 + FPC, :].T),
            "WVT": np.ascontiguousarray(Wv[f0:f0 + FPC, :].T),
            "SINB": sinb, "COSB": cosb, "SINP": sinp, "COSP": cosp,
        })

    nc = _get_nc()
    res = run_bass_kernel_spmd(nc, in_maps, list(range(NCORES)))
    global LAST_RESULTS
    LAST_RESULTS = res

    out = np.empty((L, B, E), dtype=np.float32)
    for c in range(NCORES):
        b = c // 2
        f0 = (c % 2) * FPC
        out[:, b, f0:f0 + FPC] = res.results[c]["OUT"]
    return out
